# revision 4
# baseline (speedup 1.0000x reference)
"""Bass/TRN2 kernel for nn_BMM_S8T_S8N_S8T:
    out[b,m,n] = sat_i8(round(alpha * sum_k a[b,m,k] * b[b,n,k]))
with a: (32, 2048, 64) int8, b: (32, 2048, 64) int8, alpha: f32 scalar.

Sharding: batch dim 32 -> 8 cores x 4 batches (pure data parallel, no
cross-core communication).

Per-core design notes:
  - int8 matmul is not supported by the PE; bf16 x bf16 -> f32 PSUM is exact
    for int8 operands (products < 2^16, sums of 64 < 2^24), so inputs are
    converted to bf16 (and pre-transposed to [K, seq] layout) on host.
  - The 4 local batches are stacked in pairs along SBUF partitions:
    partitions 0-63 hold batch 2p's K=64, partitions 64-127 batch 2p+1's.
    Row-tiled matmuls (row groups 0 and 64) run the two batches' K=64
    contractions concurrently on the 128x128 PE array.
  - Requant drain (PSUM f32 -> SBUF int8, scale + round-half-even + saturate)
    is the bottleneck: only VectorE and ScalarE can read PSUM (1 f32/lane/
    cycle; matmul PSUM output is f32-only on TRN2, so DVE 2x modes don't
    apply). Both engines drain in parallel on different PSUM banks, in
    [128, 1024] (2-bank) units; with 8 banks this 4-unit rotation is the
    optimal FD (bigger units stall the PE, smaller ones pay more per-instr
    overhead). Steady state is fully drain-packed; the remaining levers are
    the ramp (first drain time) and the tail (last drain -> last DMA byte):
      * ramp: small first input pieces (a0 first m-tile + b cols 0:512),
        4 short warmup matmuls, and FD-512 bank-granular early drains for
        the first m-tile so both drain engines start ~1µs earlier.
      * tail: the last tile's h=0 half is DMA'd as soon as it drains, the
        final unit pair is drained as 4 FD-512 pieces split across both
        engines, and the four final 64KB output DMAs issue from four
        different engine queues (vector/scalar/sync/gpsimd) in parallel.
  - A single tensor_scalar_mul / activation(Copy, scale) instruction does the
    whole requant bit-exactly (verified vs numpy round/clip on HW).
"""

import numpy as np
import ml_dtypes

B, M, N, K = 32, 2048, 2048, 64
NCORES = 8
BPC = B // NCORES          # batches per core (4)
MT = M // 128              # m-tiles per batch (16)
NHALF = 2                  # two 1024-col drain units per m-tile row block
UNIT = N // NHALF          # 1024 columns per drain unit

_CACHE = {}


def _build(alpha: float):
    import concourse.bacc as bacc
    import concourse.mybir as mybir
    from concourse.tile import TileContext

    bf16 = mybir.dt.bfloat16
    f32 = mybir.dt.float32
    i8 = mybir.dt.int8

    nc = bacc.Bacc("TRN2")
    aT = nc.dram_tensor("aT", (BPC // 2, 128, M), bf16, kind="ExternalInput")
    bT = nc.dram_tensor("bT", (BPC // 2, 128, N), bf16, kind="ExternalInput")
    out = nc.dram_tensor("out", (BPC, M, N), i8, kind="ExternalOutput")

    # measured drain cadences (ns) per free-dim element count
    def dve_cost(fd):
        return (fd + 64) / 0.96

    def act_cost(fd):
        return (fd + 202) / 1.2

    WARMUP_MM = 4  # short dummy matmuls to lift the PE HAM clock gate

    # scratch operand for warm-up matmuls: deliberately never written (the
    # values don't matter and the scratch PSUM bank is never read); a raw
    # (non-pool) tensor so Tile's release tracking doesn't object.
    wz = nc.alloc_sbuf_tensor("wz", [128, 512], bf16)

    with TileContext(nc) as tc:
        with (
            tc.tile_pool(name="inp", bufs=1) as inp_pool,
            tc.tile_pool(name="ps", bufs=4, space="PSUM") as psum_pool,
            tc.tile_pool(name="outp", bufs=8) as out_pool,
        ):
            wps = psum_pool.tile([128, UNIT], f32, tag="ps")
            for _ in range(WARMUP_MM):
                nc.tensor.matmul(
                    wps[:, 0:256], wz[:, 0:128], wz[:, 0:256], start=True, stop=True
                )

            # Input tiles, split by "when first needed". The sync queue takes
            # the pieces on the first-drain critical path (a's first m-tile,
            # then b cols 0:512); everything else streams on the GpSimd SWDGE
            # ring in order of first use. Sync is free again by ~9.5us for
            # the steady-state output-DMA triggers.
            a0_first = inp_pool.tile([128, 128], bf16, tag="a0f")
            bq0 = inp_pool.tile([128, 512], bf16, tag="bq0")
            b02 = inp_pool.tile([128, 512], bf16, tag="b02")
            b03 = inp_pool.tile([128, 1024], bf16, tag="b03")
            a0_r0 = inp_pool.tile([128, 896], bf16, tag="a0r0")
            a0_r1 = inp_pool.tile([128, 1024], bf16, tag="a0r1")
            a1_sb = inp_pool.tile([128, M], bf16, tag="a1")
            b1_sb = inp_pool.tile([128, N], bf16, tag="b1")
            nc.sync.dma_start(out=a0_first[:, :], in_=aT[0, :, 0:128])
            nc.sync.dma_start(out=bq0[:, :], in_=bT[0, :, 0:512])
            nc.sync.dma_start(out=b02[:, :], in_=bT[0, :, 512:1024])
            nc.sync.dma_start(out=b03[:, :], in_=bT[0, :, 1024:2048])
            nc.gpsimd.dma_start(out=a0_r0[:, :], in_=aT[0, :, 128:1024])
            nc.gpsimd.dma_start(out=a0_r1[:, :], in_=aT[0, :, 1024:M])
            nc.gpsimd.dma_start(out=a1_sb[:, :], in_=aT[1])
            nc.gpsimd.dma_start(out=b1_sb[:, :], in_=bT[1])

            def lhs_ap(p, t, rows):
                if p == 1:
                    return a1_sb[rows, 128 * t : 128 * (t + 1)]
                if t == 0:
                    return a0_first[rows, :]
                if t < 8:
                    return a0_r0[rows, 128 * (t - 1) : 128 * t]
                return a0_r1[rows, 128 * (t - 8) : 128 * (t - 7)]

            def rhs_ap(p, n0, rows):
                # 512-wide rhs slice starting at output column n0
                if p == 1:
                    return b1_sb[rows, n0 : n0 + 512]
                if n0 == 0:
                    return bq0[rows, :]
                if n0 == 512:
                    return b02[rows, :]
                return b03[rows, n0 - 1024 : n0 - 512]

            dve_t = act_t = 0.0

            def drain(o_ap, ps_ap, fd, force=None):
                """Requant one PSUM range into SBUF int8 on one engine.
                force: None = greedy balance, 'v' = VectorE, 's' = ScalarE."""
                nonlocal dve_t, act_t
                dn, an = dve_cost(fd), act_cost(fd)
                use_dve = force == "v" or (
                    force is None and dve_t + dn <= act_t + an
                )
                if use_dve:
                    nc.vector.tensor_scalar_mul(o_ap, ps_ap, alpha)
                    dve_t += dn
                else:
                    nc.scalar.activation(
                        o_ap,
                        ps_ap,
                        mybir.ActivationFunctionType.Copy,
                        scale=alpha,
                    )
                    act_t += an

            rows0, rows1 = slice(0, 64), slice(64, 128)

            # ---------------- ramp: p=0, t=0 as bank-granular pieces -------
            # A: output cols 0:512 (from bq0), FD-512 drains as soon as the
            #    first 160KB of input lands. B: cols 512:1024 (from b02).
            # D: cols 1024:2048 (from b03), standard FD-1024 unit == (0,1).
            o0_t0 = out_pool.tile([128, N], i8, tag="o", name="o0_00")
            o1_t0 = out_pool.tile([128, N], i8, tag="o", name="o1_00")
            for name, rhs, cs in (("A", bq0, slice(0, 512)), ("B", b02, slice(512, 1024))):
                psr0 = psum_pool.tile([128, UNIT], f32, tag="ps", name=f"ps0_{name}")
                psr1 = psum_pool.tile([128, UNIT], f32, tag="ps", name=f"ps1_{name}")
                nc.tensor.matmul(
                    psr0[:, 0:512], lhs_ap(0, 0, rows0), rhs[rows0, :],
                    start=True, stop=True,
                )
                nc.tensor.matmul(
                    psr1[:, 0:512], lhs_ap(0, 0, rows1), rhs[rows1, :],
                    start=True, stop=True,
                )
                drain(o0_t0[:, cs], psr0[:, 0:512], 512, force="v")
                drain(o1_t0[:, cs], psr1[:, 0:512], 512, force="s")

            # remaining work, in emission order. Entries:
            #   (p, t, h) standard FD-1024 unit pairs.
            # p=0 h=1 units for t=0..2 are deferred until b03 has landed.
            order = [(0, 1, 0), (0, 2, 0), (0, 0, 1), (0, 1, 1), (0, 2, 1)]
            order += [(0, t, h) for t in range(3, MT) for h in range(NHALF)]
            order += [(1, t, h) for t in range(MT) for h in range(NHALF)]

            otiles = {(0, 0): (o0_t0, o1_t0)}
            ndone = {(0, 0): 1}  # t=0's A+B already cover half its columns
            NEED = 2  # h-units per tile

            for p, t, h in order:
                last_tile = p == 1 and t == MT - 1
                if (p, t) not in otiles:
                    o0 = out_pool.tile([128, N], i8, tag="o", name=f"o0_{p}_{t}")
                    o1 = out_pool.tile([128, N], i8, tag="o", name=f"o1_{p}_{t}")
                    otiles[(p, t)] = (o0, o1)
                    ndone[(p, t)] = 0
                o0, o1 = otiles[(p, t)]
                ps0 = psum_pool.tile([128, UNIT], f32, tag="ps", name=f"ps0_{p}_{t}_{h}")
                ps1 = psum_pool.tile([128, UNIT], f32, tag="ps", name=f"ps1_{p}_{t}_{h}")
                for j in range(2):  # 512-col matmul within unit
                    n0 = UNIT * h + 512 * j
                    c = slice(512 * j, 512 * (j + 1))
                    nc.tensor.matmul(
                        ps0[:, c], lhs_ap(p, t, rows0), rhs_ap(p, n0, rows0),
                        start=True, stop=True,
                    )
                    nc.tensor.matmul(
                        ps1[:, c], lhs_ap(p, t, rows1), rhs_ap(p, n0, rows1),
                        start=True, stop=True,
                    )
                hs = slice(UNIT * h, UNIT * (h + 1))
                if last_tile and h == 1:
                    # tail: 4 FD-512 drains split across both engines, each
                    # followed by its own 64KB DMA on a distinct queue so the
                    # final transfers overlap instead of serializing on sync.
                    drain(o0[:, 1024:1536], ps0[:, 0:512], 512, force="v")
                    drain(o1[:, 1024:1536], ps1[:, 0:512], 512, force="s")
                    drain(o1[:, 1536:2048], ps1[:, 512:1024], 512, force="v")
                    drain(o0[:, 1536:2048], ps0[:, 512:1024], 512, force="s")
                    mrows = slice(128 * t, 128 * (t + 1))
                    nc.sync.dma_start(
                        out=out[2 * p, mrows, 1024:1536], in_=o0[:, 1024:1536]
                    )
                    nc.gpsimd.dma_start(
                        out=out[2 * p + 1, mrows, 1024:1536], in_=o1[:, 1024:1536]
                    )
                    nc.sync.dma_start(
                        out=out[2 * p + 1, mrows, 1536:2048], in_=o1[:, 1536:2048]
                    )
                    nc.scalar.dma_start(
                        out=out[2 * p, mrows, 1536:2048], in_=o0[:, 1536:2048]
                    )
                    continue
                drain(o0[:, hs], ps0[:, :], UNIT)
                drain(o1[:, hs], ps1[:, :], UNIT)
                if last_tile and h == 0:
                    # ship the lower halves while h=1 is still draining
                    mrows = slice(128 * t, 128 * (t + 1))
                    nc.sync.dma_start(out=out[2 * p, mrows, 0:1024], in_=o0[:, 0:1024])
                    nc.gpsimd.dma_start(
                        out=out[2 * p + 1, mrows, 0:1024], in_=o1[:, 0:1024]
                    )
                    continue
                ndone[(p, t)] += 1
                if ndone[(p, t)] < NEED:
                    continue
                for which, o in ((0, o0), (1, o1)):
                    nc.sync.dma_start(
                        out=out[2 * p + which, 128 * t : 128 * (t + 1), :],
                        in_=o[:, :],
                    )
    nc.compile()
    return nc


def kernel(a: np.ndarray, b: np.ndarray, alpha) -> np.ndarray:
    from concourse.bass_utils import run_bass_kernel_spmd

    a = np.asarray(a)
    b = np.asarray(b)
    alpha_f = float(np.asarray(alpha))

    key = alpha_f
    if key not in _CACHE:
        _CACHE[key] = _build(alpha_f)
    nc = _CACHE[key]

    # host-side layout prep: per batch, [seq, K] int8 -> [K, seq] bf16, then
    # stack batch pairs along the partition axis.
    aT = np.ascontiguousarray(a.transpose(0, 2, 1)).astype(ml_dtypes.bfloat16)
    bT = np.ascontiguousarray(b.transpose(0, 2, 1)).astype(ml_dtypes.bfloat16)
    aT = aT.reshape(NCORES, BPC // 2, 128, M)
    bT = bT.reshape(NCORES, BPC // 2, 128, N)

    in_maps = [{"aT": aT[c], "bT": bT[c]} for c in range(NCORES)]
    try:
        res = run_bass_kernel_spmd(nc, in_maps, core_ids=list(range(NCORES)))
    except Exception:
        # one retry in case a previous process left a device in a bad state
        res = run_bass_kernel_spmd(nc, in_maps, core_ids=list(range(NCORES)))
    outs = [res.results[c]["out"] for c in range(NCORES)]
    return np.concatenate(outs, axis=0).astype(np.int8)
